# revision 4
# baseline (speedup 1.0000x reference)
"""GE2E-style speaker-verification loss on 8 Trainium2 NeuronCores.

Math (per batch element b, handled by one core):
    c[k]      = mean_i e[k,i,:]                       (group centroid)
    raw[n,k]  = <e_n, c_k>          n = (j,i) flattened
    S         = w*raw + b,  diag (k==j) replaced by the leave-one-out value
    S_self    = w*(M*dot_own - ||e_n||^2)/(M-1) + b
    loss      = sum_n logsumexp_k S[n,:] - sum_n S_self[n]

The +b bias cancels between the two terms, and w is folded into the
centroid selector (w/M = 0.625, exact in fp32), so the device only ever
sees w-scaled quantities:
    term_n = WM2 + ln(sumexp_cross + exp(wself - WM2)) - wself
    WM2    = max(row max excl. diag, wself)
The diagonal is excluded from max/sumexp exactly by a third rank-8
matmul that accumulates -1e6 onto the diagonal positions of each PSUM
similarity tile (diag dominates the row statistically, so a
subtract-after-exp correction would catastrophically cancel).
"""

import sys

sys.path.insert(0, "/opt/trn_rl_repo")

from contextlib import ExitStack

import numpy as np

import concourse.bass as bass  # noqa: F401  (engine types referenced via nc)
import concourse.mybir as mybir
from concourse import bacc, tile

F32 = mybir.dt.float32
AF = mybir.ActivationFunctionType
ALU = mybir.AluOpType
AX = mybir.AxisListType

B, N, M, D = 8, 256, 16, 256
ROWS = N * M            # 4096 rows per core
NT = ROWS // 128        # 32 n-tiles of 128 rows
GPT = 128 // M          # 8 speaker groups per tile
NCORES = 8
BIG = 1.0e6


def _host_consts(w):
    r = np.arange(128)
    ident = np.eye(128, dtype=np.float32)
    # centroid selector with w/M folded in (0.625 exact for w=10)
    sel = np.zeros((128, GPT), np.float32)
    sel[r, r // M] = np.float32(w) / np.float32(M)
    # mask8 transposed: stationary of the diag-kill matmul
    m8t = np.zeros((GPT, 128), np.float32)
    m8t[r // M, r] = 1.0
    # per-row one-hot of own group, tiled over the 32 n-tiles
    mask8 = np.zeros((128, GPT), np.float32)
    mask8[r, r // M] = 1.0
    mask_full = np.tile(mask8, (1, NT))
    # shifted -BIG pattern: H[:, 248-8t : 504-8t][g,k] == -BIG iff k == 8t+g
    H = np.zeros((GPT, 504), np.float32)
    H[np.arange(GPT), 248 + np.arange(GPT)] = -BIG
    return ident, sel, m8t, mask_full, H


def _body(tc, emb, ident_d, sel_d, m8t_d, mfull_d, H_d, loss_d, w):
    nc = tc.nc
    with ExitStack() as ctx:
        const = ctx.enter_context(tc.tile_pool(name="const", bufs=1))
        pers = ctx.enter_context(tc.tile_pool(name="pers", bufs=1))
        e_pool = ctx.enter_context(tc.tile_pool(name="e", bufs=3))
        dump = ctx.enter_context(tc.tile_pool(name="dump", bufs=2))
        tailp = ctx.enter_context(tc.tile_pool(name="tail", bufs=1))
        ps_diag_p = ctx.enter_context(tc.tile_pool(name="psdg", bufs=1, space="PSUM"))

        ident = const.tile([128, 128], F32, tag="ident")
        nc.sync.dma_start(ident[:], ident_d)
        sel = const.tile([128, GPT], F32, tag="sel")
        nc.sync.dma_start(sel[:], sel_d)
        m8t = const.tile([GPT, 128], F32, tag="m8t")
        nc.sync.dma_start(m8t[:], m8t_d)
        mfull = const.tile([128, NT * GPT], F32, tag="mfull")
        nc.sync.dma_start(mfull[:], mfull_d)
        Ht = const.tile([GPT, 504], F32, tag="H")
        nc.sync.dma_start(Ht[:], H_d)
        ones = const.tile([128, 1], F32, tag="ones")
        nc.vector.memset(ones[:], 1.0)

        eT0 = pers.tile([128, ROWS], F32, tag="eT0")
        eT1 = pers.tile([128, ROWS], F32, tag="eT1")
        sq_col = pers.tile([128, NT], F32, tag="sq")
        negm = pers.tile([128, NT], F32, tag="negm")
        sumexp = pers.tile([128, NT], F32, tag="sumexp")
        ct = [pers.tile([128, N], F32, tag=f"ct{i}", name=f"ct{i}")
              for i in range(2)]

        ps_diag = ps_diag_p.tile([128, NT * GPT], F32, tag="psdiag")

        # ---- Stage A: load, squares, centroids, transpose e -> eT ----
        with tc.tile_pool(name="psA", bufs=1, space="PSUM") as psA, \
             tc.tile_pool(name="pstp", bufs=2, space="PSUM") as pstp:
            pct = [psA.tile([128, N], F32, tag=f"pct{i}", name=f"pct{i}")
                   for i in range(2)]
            for q in range(NT // 4):
                e_big = e_pool.tile([128, 4 * D], F32, tag="ebig")
                src = emb[q * 512:(q + 1) * 512, :].rearrange(
                    "(a p) d -> p a d", p=128)
                nc.sync.dma_start(
                    e_big[:].rearrange("p (a d) -> p a d", d=D), src)
                tp0 = pstp.tile([128, 512], F32, tag="tp0")
                tp1 = pstp.tile([128, 512], F32, tag="tp1")
                for j in range(4):
                    t = 4 * q + j
                    ej = e_big[:, j * D:(j + 1) * D]
                    sdump = dump.tile([128, D], F32, tag="dump")
                    nc.scalar.activation(sdump[:], ej, AF.Square,
                                         accum_out=sq_col[:, t:t + 1])
                    for h in range(2):
                        ejh = e_big[:, j * D + h * 128:j * D + (h + 1) * 128]
                        tph = (tp0, tp1)[h]
                        nc.tensor.transpose(tph[:, j * 128:(j + 1) * 128],
                                            ejh, ident[:])
                        # centroid columns: out[d, g] = sum_nm e[nm,d]*sel[nm,g]
                        nc.tensor.matmul(
                            pct[h][:, t * GPT:(t + 1) * GPT],
                            lhsT=ejh, rhs=sel[:], start=True, stop=True)
                nc.vector.tensor_copy(eT0[:, q * 512:(q + 1) * 512], tp0[:])
                nc.vector.tensor_copy(eT1[:, q * 512:(q + 1) * 512], tp1[:])

            # ---- Stage B: centroid columns PSUM -> SBUF ----
            nc.vector.tensor_copy(ct[0][:], pct[0][:])
            nc.vector.tensor_copy(ct[1][:], pct[1][:])

        # ---- Stage C: similarities, diag-kill, row max, exp+rowsum ----
        ps_main_p = ctx.enter_context(
            tc.tile_pool(name="psmn", bufs=2, space="PSUM"))
        ps_loss_p = ctx.enter_context(
            tc.tile_pool(name="psls", bufs=1, space="PSUM"))
        for g in range(NT // 4):
            ps = ps_main_p.tile([128, 4 * N], F32, tag="ps")
            for j in range(4):
                t = 4 * g + j
                sub = ps[:, j * N:(j + 1) * N]
                dsl = ps_diag[:, t * GPT:(t + 1) * GPT]
                et0 = eT0[:, t * 128:(t + 1) * 128]
                et1 = eT1[:, t * 128:(t + 1) * 128]
                nc.tensor.matmul(sub, lhsT=et0, rhs=ct[0][:],
                                 start=True, stop=False, skip_group_check=True)
                nc.tensor.matmul(dsl, lhsT=et0,
                                 rhs=ct[0][:, t * GPT:(t + 1) * GPT],
                                 start=True, stop=False, skip_group_check=True)
                nc.tensor.matmul(sub, lhsT=et1, rhs=ct[1][:],
                                 start=False, stop=False,
                                 skip_group_check=True)
                nc.tensor.matmul(dsl, lhsT=et1,
                                 rhs=ct[1][:, t * GPT:(t + 1) * GPT],
                                 start=False, stop=True, skip_group_check=True)
                nc.tensor.matmul(sub, lhsT=m8t[:],
                                 rhs=Ht[:, 248 - t * GPT:504 - t * GPT],
                                 start=False, stop=True, skip_group_check=True)
            nc.vector.reduce_max(
                negm[:, g * 4:(g + 1) * 4],
                ps[:].rearrange("p (s k) -> p s k", k=N),
                axis=AX.X, negate=True)
            for j in range(4):
                t = 4 * g + j
                edump = dump.tile([128, N], F32, tag="dump")
                nc.scalar.activation(edump[:], ps[:, j * N:(j + 1) * N],
                                     AF.Exp, bias=negm[:, t:t + 1], scale=1.0,
                                     accum_out=sumexp[:, t:t + 1])

        # ---- Tail: batched [128,32] epilogue ----
        def tl(tag):
            return tailp.tile([128, NT], F32, tag=tag, name=tag)

        tmpd = tailp.tile([128, NT * GPT], F32, tag="tmpd")
        nc.vector.tensor_tensor(tmpd[:], ps_diag[:], mfull[:], op=ALU.mult)
        wdot = tl("wdot")
        nc.vector.reduce_sum(
            wdot[:], tmpd[:].rearrange("p (t g) -> p t g", g=GPT), axis=AX.X)
        t16 = tl("t16")
        nc.vector.tensor_scalar_mul(t16[:], wdot[:], float(M) / (M - 1))
        t2 = tl("t2")
        nc.vector.tensor_scalar_mul(t2[:], sq_col[:], float(w) / (M - 1))
        wself = tl("wself")
        nc.vector.tensor_tensor(wself[:], t16[:], t2[:], op=ALU.subtract)
        wm = tl("wm")
        nc.vector.tensor_scalar_mul(wm[:], negm[:], -1.0)
        wm2 = tl("wm2")
        nc.vector.tensor_tensor(wm2[:], wm[:], wself[:], op=ALU.max)
        d1 = tl("d1")
        nc.vector.tensor_tensor(d1[:], wm[:], wm2[:], op=ALU.subtract)
        e1 = tl("e1")
        nc.scalar.activation(e1[:], d1[:], AF.Exp)
        a = tl("a")
        nc.vector.tensor_tensor(a[:], sumexp[:], e1[:], op=ALU.mult)
        d3 = tl("d3")
        nc.vector.tensor_tensor(d3[:], wself[:], wm2[:], op=ALU.subtract)
        e3 = tl("e3")
        nc.scalar.activation(e3[:], d3[:], AF.Exp)
        se = tl("se")
        nc.vector.tensor_tensor(se[:], a[:], e3[:], op=ALU.add)
        lns = tl("lns")
        nc.scalar.activation(lns[:], se[:], AF.Ln)
        s1 = tl("s1")
        nc.vector.tensor_tensor(s1[:], wm2[:], lns[:], op=ALU.add)
        terms = tl("terms")
        nc.vector.tensor_tensor(terms[:], s1[:], wself[:], op=ALU.subtract)
        acc = tailp.tile([128, 1], F32, tag="acc")
        nc.vector.reduce_sum(acc[:], terms[:], axis=AX.X)
        ps_l = ps_loss_p.tile([1, 1], F32, tag="psl")
        nc.tensor.matmul(ps_l[:], lhsT=acc[:], rhs=ones[:],
                         start=True, stop=True)
        loss_sb = tailp.tile([1, 1], F32, tag="losssb")
        nc.vector.tensor_copy(loss_sb[:], ps_l[:])
        nc.sync.dma_start(loss_d, loss_sb[:])


def build_program(w):
    nc = bacc.Bacc("TRN2", target_bir_lowering=False, debug=False)
    emb = nc.dram_tensor("emb", [ROWS, D], F32, kind="ExternalInput").ap()
    ident_d = nc.dram_tensor("ident", [128, 128], F32,
                             kind="ExternalInput").ap()
    sel_d = nc.dram_tensor("sel", [128, GPT], F32, kind="ExternalInput").ap()
    m8t_d = nc.dram_tensor("mask8T", [GPT, 128], F32,
                           kind="ExternalInput").ap()
    mfull_d = nc.dram_tensor("mask_full", [128, NT * GPT], F32,
                             kind="ExternalInput").ap()
    H_d = nc.dram_tensor("H", [GPT, 504], F32, kind="ExternalInput").ap()
    loss_d = nc.dram_tensor("loss", [1, 1], F32, kind="ExternalOutput").ap()
    with tile.TileContext(nc) as tc:
        _body(tc, emb, ident_d, sel_d, m8t_d, mfull_d, H_d, loss_d, w)
    nc.compile()
    return nc


_CACHE = {}


def _get_program(w):
    key = float(w)
    if key not in _CACHE:
        _CACHE[key] = build_program(key)
    return _CACHE[key]


def make_in_maps(embeddings, w):
    ident, sel, m8t, mask_full, H = _host_consts(float(w))
    consts = {"ident": ident, "sel": sel, "mask8T": m8t,
              "mask_full": mask_full, "H": H}
    return [
        {"emb": np.ascontiguousarray(
            embeddings[c].reshape(ROWS, D).astype(np.float32)), **consts}
        for c in range(NCORES)
    ]


def kernel(embeddings, w, b):
    embeddings = np.asarray(embeddings, dtype=np.float32)
    assert embeddings.shape == (B, N, M, D), embeddings.shape
    nc = _get_program(float(w))
    in_maps = make_in_maps(embeddings, w)
    from concourse.bass_utils import run_bass_kernel_spmd
    res = run_bass_kernel_spmd(nc, in_maps, core_ids=list(range(NCORES)))
    total = np.float64(0.0)
    for r in res.results:
        total += np.float64(r["loss"][0, 0])
    # b cancels between logsumexp and self terms; only w is used on device
    return np.float32(total)


# revision 5
# speedup vs baseline: 2.4247x; 2.4247x over previous
"""GE2E-style speaker-verification loss on 8 Trainium2 NeuronCores.

Math (per batch element b, handled by one core):
    c[k]      = mean_i e[k,i,:]                       (group centroid)
    raw[n,k]  = <e_n, c_k>          n = (j,i) flattened
    S         = w*raw + b,  diag (k==j) replaced by the leave-one-out value
    S_self    = w*(M*dot_own - ||e_n||^2)/(M-1) + b
    loss      = sum_n logsumexp_k S[n,:] - sum_n S_self[n]

The +b bias cancels between the two terms, and w is folded into the
centroid selector (w/M = 0.625, exact in fp32), so the device only ever
sees w-scaled quantities:
    term_n = WM2 + ln(sumexp_cross + exp(wself - WM2)) - wself
    WM2    = max(row max excl. diag, wself)
The diagonal is excluded from max/sumexp exactly by a third rank-8
matmul that accumulates -1e6 onto the diagonal positions of each PSUM
similarity tile (diag dominates the row statistically, so a
subtract-after-exp correction would catastrophically cancel).
"""

import sys

sys.path.insert(0, "/opt/trn_rl_repo")

from contextlib import ExitStack

import numpy as np

import concourse.bass as bass  # noqa: F401  (engine types referenced via nc)
import concourse.mybir as mybir
from concourse import bacc, tile

F32 = mybir.dt.float32
BF16 = mybir.dt.bfloat16
AF = mybir.ActivationFunctionType
ALU = mybir.AluOpType
AX = mybir.AxisListType

B, N, M, D = 8, 256, 16, 256
ROWS = N * M            # 4096 rows per core
NT = ROWS // 128        # 32 n-tiles of 128 rows
GPT = 128 // M          # 8 speaker groups per tile
NCORES = 8
BIG = 1.0e6


def _host_consts(w):
    import ml_dtypes
    bf = ml_dtypes.bfloat16
    r = np.arange(128)
    ident = np.eye(128, dtype=bf)
    # centroid selector with w/M folded in (0.625 exact for w=10)
    sel = np.zeros((128, GPT), bf)
    sel[r, r // M] = bf(np.float32(w) / np.float32(M))
    # mask8 transposed: stationary of the diag-kill matmul
    m8t = np.zeros((GPT, 128), bf)
    m8t[r // M, r] = 1.0
    # per-row one-hot of own group, tiled over the 32 n-tiles
    mask8 = np.zeros((128, GPT), np.float32)
    mask8[r, r // M] = 1.0
    mask_full = np.tile(mask8, (1, NT))
    # shifted -BIG pattern: H[:, 248-8t : 504-8t][g,k] == -BIG iff k == 8t+g
    H = np.zeros((GPT, 504), bf)
    H[np.arange(GPT), 248 + np.arange(GPT)] = bf(-BIG)
    return ident, sel, m8t, mask_full, H


def _body(tc, emb, ident_d, sel_d, m8t_d, mfull_d, H_d, loss_d, w):
    nc = tc.nc
    with ExitStack() as ctx:
        const = ctx.enter_context(tc.tile_pool(name="const", bufs=1))
        pers = ctx.enter_context(tc.tile_pool(name="pers", bufs=1))
        e_pool = ctx.enter_context(tc.tile_pool(name="e", bufs=3))
        dump = ctx.enter_context(tc.tile_pool(name="dump", bufs=2))
        tailp = ctx.enter_context(tc.tile_pool(name="tail", bufs=1))
        ps_diag_p = ctx.enter_context(tc.tile_pool(name="psdg", bufs=1, space="PSUM"))

        ident = const.tile([128, 128], BF16, tag="ident")
        nc.sync.dma_start(ident[:], ident_d)
        sel = const.tile([128, GPT], BF16, tag="sel")
        nc.sync.dma_start(sel[:], sel_d)
        m8t = const.tile([GPT, 128], BF16, tag="m8t")
        nc.sync.dma_start(m8t[:], m8t_d)
        mfull = const.tile([128, NT * GPT], F32, tag="mfull")
        nc.sync.dma_start(mfull[:], mfull_d)
        Ht = const.tile([GPT, 504], BF16, tag="H")
        nc.sync.dma_start(Ht[:], H_d)
        ones = const.tile([128, 1], F32, tag="ones")
        nc.vector.memset(ones[:], 1.0)

        eT0 = pers.tile([128, ROWS], BF16, tag="eT0")
        eT1 = pers.tile([128, ROWS], BF16, tag="eT1")
        sq_col = pers.tile([128, NT], F32, tag="sq")
        negm = pers.tile([128, NT], F32, tag="negm")
        sumexp = pers.tile([128, NT], F32, tag="sumexp")
        ct = [pers.tile([128, N], BF16, tag=f"ct{i}", name=f"ct{i}")
              for i in range(2)]

        ps_diag = ps_diag_p.tile([128, NT * GPT], F32, tag="psdiag")

        # ---- Stage A: load, squares, centroids, transpose e -> eT ----
        with tc.tile_pool(name="psA", bufs=1, space="PSUM") as psA, \
             tc.tile_pool(name="pstp", bufs=2, space="PSUM") as pstp:
            pct = [psA.tile([128, N], F32, tag=f"pct{i}", name=f"pct{i}")
                   for i in range(2)]
            for q in range(NT // 4):
                e_big = e_pool.tile([128, 4 * D], F32, tag="ebig")
                src = emb[q * 512:(q + 1) * 512, :].rearrange(
                    "(a p) d -> p a d", p=128)
                nc.sync.dma_start(
                    e_big[:].rearrange("p (a d) -> p a d", d=D), src)
                e_bf = e_pool.tile([128, 4 * D], BF16, tag="ebf")
                nc.vector.tensor_copy(e_bf[:], e_big[:])
                tp0 = pstp.tile([128, 512], BF16, tag="tp0")
                tp1 = pstp.tile([128, 512], BF16, tag="tp1")
                for j in range(4):
                    t = 4 * q + j
                    ej = e_big[:, j * D:(j + 1) * D]
                    sdump = dump.tile([128, D], F32, tag="dump")
                    nc.scalar.activation(sdump[:], ej, AF.Square,
                                         accum_out=sq_col[:, t:t + 1])
                    for h in range(2):
                        ejh = e_bf[:, j * D + h * 128:j * D + (h + 1) * 128]
                        tph = (tp0, tp1)[h]
                        nc.tensor.transpose(tph[:, j * 128:(j + 1) * 128],
                                            ejh, ident[:])
                        # centroid columns: out[d, g] = sum_nm e[nm,d]*sel[nm,g]
                        nc.tensor.matmul(
                            pct[h][:, t * GPT:(t + 1) * GPT],
                            lhsT=ejh, rhs=sel[:], start=True, stop=True)
                nc.vector.tensor_copy(eT0[:, q * 512:(q + 1) * 512], tp0[:])
                nc.vector.tensor_copy(eT1[:, q * 512:(q + 1) * 512], tp1[:])

            # ---- Stage B: centroid columns PSUM -> SBUF ----
            nc.vector.tensor_copy(ct[0][:], pct[0][:])
            nc.vector.tensor_copy(ct[1][:], pct[1][:])

        # ---- Stage C: similarities, diag-kill, row max, exp+rowsum ----
        ps_main_p = ctx.enter_context(
            tc.tile_pool(name="psmn", bufs=2, space="PSUM"))
        ps_loss_p = ctx.enter_context(
            tc.tile_pool(name="psls", bufs=1, space="PSUM"))
        for g in range(NT // 4):
            ps = ps_main_p.tile([128, 4 * N], F32, tag="ps")
            for j in range(4):
                t = 4 * g + j
                sub = ps[:, j * N:(j + 1) * N]
                dsl = ps_diag[:, t * GPT:(t + 1) * GPT]
                et0 = eT0[:, t * 128:(t + 1) * 128]
                et1 = eT1[:, t * 128:(t + 1) * 128]
                nc.tensor.matmul(sub, lhsT=et0, rhs=ct[0][:],
                                 start=True, stop=False, skip_group_check=True)
                nc.tensor.matmul(dsl, lhsT=et0,
                                 rhs=ct[0][:, t * GPT:(t + 1) * GPT],
                                 start=True, stop=False, skip_group_check=True)
                nc.tensor.matmul(sub, lhsT=et1, rhs=ct[1][:],
                                 start=False, stop=False,
                                 skip_group_check=True)
                nc.tensor.matmul(dsl, lhsT=et1,
                                 rhs=ct[1][:, t * GPT:(t + 1) * GPT],
                                 start=False, stop=True, skip_group_check=True)
                nc.tensor.matmul(sub, lhsT=m8t[:],
                                 rhs=Ht[:, 248 - t * GPT:504 - t * GPT],
                                 start=False, stop=True, skip_group_check=True)
            nc.vector.reduce_max(
                negm[:, g * 4:(g + 1) * 4],
                ps[:].rearrange("p (s k) -> p s k", k=N),
                axis=AX.X, negate=True)
            for j in range(4):
                t = 4 * g + j
                edump = dump.tile([128, N], F32, tag="dump")
                nc.scalar.activation(edump[:], ps[:, j * N:(j + 1) * N],
                                     AF.Exp, bias=negm[:, t:t + 1], scale=1.0,
                                     accum_out=sumexp[:, t:t + 1])

        # ---- Tail: batched [128,32] epilogue ----
        def tl(tag):
            return tailp.tile([128, NT], F32, tag=tag, name=tag)

        tmpd = tailp.tile([128, NT * GPT], F32, tag="tmpd")
        nc.vector.tensor_tensor(tmpd[:], ps_diag[:], mfull[:], op=ALU.mult)
        wdot = tl("wdot")
        nc.vector.reduce_sum(
            wdot[:], tmpd[:].rearrange("p (t g) -> p t g", g=GPT), axis=AX.X)
        t16 = tl("t16")
        nc.vector.tensor_scalar_mul(t16[:], wdot[:], float(M) / (M - 1))
        t2 = tl("t2")
        nc.vector.tensor_scalar_mul(t2[:], sq_col[:], float(w) / (M - 1))
        wself = tl("wself")
        nc.vector.tensor_tensor(wself[:], t16[:], t2[:], op=ALU.subtract)
        wm = tl("wm")
        nc.vector.tensor_scalar_mul(wm[:], negm[:], -1.0)
        wm2 = tl("wm2")
        nc.vector.tensor_tensor(wm2[:], wm[:], wself[:], op=ALU.max)
        d1 = tl("d1")
        nc.vector.tensor_tensor(d1[:], wm[:], wm2[:], op=ALU.subtract)
        e1 = tl("e1")
        nc.scalar.activation(e1[:], d1[:], AF.Exp)
        a = tl("a")
        nc.vector.tensor_tensor(a[:], sumexp[:], e1[:], op=ALU.mult)
        d3 = tl("d3")
        nc.vector.tensor_tensor(d3[:], wself[:], wm2[:], op=ALU.subtract)
        e3 = tl("e3")
        nc.scalar.activation(e3[:], d3[:], AF.Exp)
        se = tl("se")
        nc.vector.tensor_tensor(se[:], a[:], e3[:], op=ALU.add)
        lns = tl("lns")
        nc.scalar.activation(lns[:], se[:], AF.Ln)
        s1 = tl("s1")
        nc.vector.tensor_tensor(s1[:], wm2[:], lns[:], op=ALU.add)
        terms = tl("terms")
        nc.vector.tensor_tensor(terms[:], s1[:], wself[:], op=ALU.subtract)
        acc = tailp.tile([128, 1], F32, tag="acc")
        nc.vector.reduce_sum(acc[:], terms[:], axis=AX.X)
        ps_l = ps_loss_p.tile([1, 1], F32, tag="psl")
        nc.tensor.matmul(ps_l[:], lhsT=acc[:], rhs=ones[:],
                         start=True, stop=True)
        loss_sb = tailp.tile([1, 1], F32, tag="losssb")
        nc.vector.tensor_copy(loss_sb[:], ps_l[:])
        nc.sync.dma_start(loss_d, loss_sb[:])


def build_program(w):
    nc = bacc.Bacc("TRN2", target_bir_lowering=False, debug=False)
    emb = nc.dram_tensor("emb", [ROWS, D], F32, kind="ExternalInput").ap()
    ident_d = nc.dram_tensor("ident", [128, 128], BF16,
                             kind="ExternalInput").ap()
    sel_d = nc.dram_tensor("sel", [128, GPT], BF16, kind="ExternalInput").ap()
    m8t_d = nc.dram_tensor("mask8T", [GPT, 128], BF16,
                           kind="ExternalInput").ap()
    mfull_d = nc.dram_tensor("mask_full", [128, NT * GPT], F32,
                             kind="ExternalInput").ap()
    H_d = nc.dram_tensor("H", [GPT, 504], BF16, kind="ExternalInput").ap()
    loss_d = nc.dram_tensor("loss", [1, 1], F32, kind="ExternalOutput").ap()
    with tile.TileContext(nc) as tc:
        _body(tc, emb, ident_d, sel_d, m8t_d, mfull_d, H_d, loss_d, w)
    nc.compile()
    return nc


_CACHE = {}


def _get_program(w):
    key = float(w)
    if key not in _CACHE:
        _CACHE[key] = build_program(key)
    return _CACHE[key]


def make_in_maps(embeddings, w):
    ident, sel, m8t, mask_full, H = _host_consts(float(w))
    consts = {"ident": ident, "sel": sel, "mask8T": m8t,
              "mask_full": mask_full, "H": H}
    return [
        {"emb": np.ascontiguousarray(
            embeddings[c].reshape(ROWS, D).astype(np.float32)), **consts}
        for c in range(NCORES)
    ]


def kernel(embeddings, w, b):
    embeddings = np.asarray(embeddings, dtype=np.float32)
    assert embeddings.shape == (B, N, M, D), embeddings.shape
    nc = _get_program(float(w))
    in_maps = make_in_maps(embeddings, w)
    from concourse.bass_utils import run_bass_kernel_spmd
    res = run_bass_kernel_spmd(nc, in_maps, core_ids=list(range(NCORES)))
    total = np.float64(0.0)
    for r in res.results:
        total += np.float64(r["loss"][0, 0])
    # b cancels between logsumexp and self terms; only w is used on device
    return np.float32(total)
